# revision 5
# baseline (speedup 1.0000x reference)
# Multi-head causal attention (B=2, S=2048, D=768, H=12) on 8 Trainium2 cores.
#
# Sharding: (batch, head)-parallel. Each core owns 3 heads of one batch and
# computes q/k/v projections for those heads, causal attention, and its
# 3-head slice of the output projection as a partial sum [S, D]. The host
# sums the four partials per batch (the output projection is linear in the
# concatenated heads) -- no on-device collectives needed, and every core runs
# an identical instruction stream (SPMD-clean), with per-core behavior
# carried entirely by the input data.
#
# On-chip layout (all matmul operands bf16, accumulation fp32 in PSUM):
#   x.T tiles arrive via HWDGE DMA-transpose from host-precast bf16 arrays.
#   Q.T,K.T: [dh, s] so scores.T = K.T_tile.T @ Q.T needs no transposes.
#   V': [s, 65*hh] with a ones column per head -> attn.T = V'.T @ P.T gives
#       the softmax denominator as row 64 for free.
#   softmax skips max-subtraction (scores ~ N(0,1); exp is safe in fp32) and
#   is fused into one ScalarE exp op per PSUM group of score tiles.
#   Causal masking: host inspects the runtime mask, classifies 128x256 tiles
#   as dead (skipped), clean, or masked (additive bf16 block applied via an
#   identity-matmul accumulate into the scores PSUM group).

import os
import numpy as np
import ml_dtypes

import concourse.bass as bass
import concourse.bacc as bacc
import concourse.mybir as mybir
import concourse.tile as tile
from concourse.bass_utils import run_bass_kernel_spmd
from concourse.masks import make_identity

BF16 = ml_dtypes.bfloat16
bf16 = mybir.dt.bfloat16
f32 = mybir.dt.float32

B, S, D, H = 2, 2048, 768, 12
DH = D // H            # 64
NCORES = 8
HPC = 3                # heads per core
QP = 256               # q-pair width (softmax column group)
NP = S // QP           # 8 pairs
KT = 128               # key tile
NKT = S // KT          # 16 key tiles
MASK_NEG = -30000.0    # effectively -inf after exp, exact 0.0 in fp32


def _classify_mask(mask):
    """Host-side mask inspection -> per-(pair, ktile) schedule + mask blocks.

    Returns (live, blocks): live[p] is a list of (ktile, block_idx|None);
    blocks is [n, KT, QP] bf16 additive mask blocks (transposed to [k, q]).
    The schedule is global (mask broadcasts over batch/head), so the SPMD
    program is identical on every core.
    """
    m = np.asarray(mask, np.float32).reshape(S, S)  # [q, k]
    # Blocks are added to raw q@k.T scores, *before* the fused exp(x/sqrt(DH))
    # scale, so pre-multiply by sqrt(DH) to match reference's mask * -1e9.
    add = np.clip(m * np.float32(-1e9) * np.float32(np.sqrt(DH)),
                  MASK_NEG, -MASK_NEG).astype(np.float32)
    alive = add > (MASK_NEG + 1.0)
    if not alive.any(axis=1).all():
        # A fully-masked query row: softmax is degenerate; keep every tile
        # live so we at least match reference up to its own degeneracy.
        alive = np.ones_like(alive)
    live = []
    blocks = []
    bkey = {}
    for p in range(NP):
        qs = slice(QP * p, QP * (p + 1))
        lst = []
        for t in range(NKT):
            ks = slice(KT * t, KT * (t + 1))
            if not alive[qs, ks].any():
                continue  # dead tile: every entry masked away
            blk = add[qs, ks]
            if np.all(blk == 0.0):
                lst.append((t, None))
            else:
                tb = np.ascontiguousarray(blk.T).astype(BF16)  # [k, q]
                key = tb.tobytes()
                if key not in bkey:
                    bkey[key] = len(blocks)
                    blocks.append(tb)
                lst.append((t, bkey[key]))
        assert lst, "pair with no live key tiles"
        live.append(lst)
    if blocks:
        blocks = np.stack(blocks)
    else:
        blocks = np.zeros((1, KT, QP), BF16)
    return live, blocks


def _build_program(live, n_blocks):
    """Emit the SPMD Bass program (same for all cores)."""
    nc = bacc.Bacc("TRN2", target_bir_lowering=False, debug=False,
                   num_devices=NCORES)

    xq_d = nc.dram_tensor("xq", [S, D], bf16, kind="ExternalInput").ap()
    xk_d = nc.dram_tensor("xk", [S, D], bf16, kind="ExternalInput").ap()
    xv_d = nc.dram_tensor("xv", [S, D], bf16, kind="ExternalInput").ap()
    wq_d = nc.dram_tensor("wq3", [D, HPC * DH], bf16, kind="ExternalInput").ap()
    wk_d = nc.dram_tensor("wk3", [D, HPC * DH], bf16, kind="ExternalInput").ap()
    wv_d = nc.dram_tensor("wv3i", [D, HPC * 65], bf16, kind="ExternalInput").ap()
    bq_d = nc.dram_tensor("bq3", [HPC * DH], f32, kind="ExternalInput").ap()
    bk_d = nc.dram_tensor("bk3", [HPC * DH], f32, kind="ExternalInput").ap()
    bv_d = nc.dram_tensor("bv3i", [1, HPC * 65], bf16, kind="ExternalInput").ap()
    wco_d = nc.dram_tensor("wco", [HPC * DH + 1, D], bf16, kind="ExternalInput").ap()
    mk_d = nc.dram_tensor("maskt", [n_blocks, KT, QP], bf16, kind="ExternalInput").ap()
    out_d = nc.dram_tensor("partial", [S, D], f32, kind="ExternalOutput").ap()

    Exp = mybir.ActivationFunctionType.Exp
    GRP = 4  # score ktiles per PSUM exp group ([128, 1024] = 2 banks)

    with tile.TileContext(nc) as tc:
        with tc.tile_pool(name="const", bufs=1) as cp, \
             tc.tile_pool(name="work", bufs=1) as wp, \
             tc.tile_pool(name="flow", bufs=3) as fp:

            # ---- constants / inputs to SBUF ----
            wq_sb = cp.tile([128, 6, HPC * DH], bf16)
            nc.sync.dma_start(wq_sb[:], wq_d.rearrange("(a p) n -> p a n", p=128))
            wk_sb = cp.tile([128, 6, HPC * DH], bf16)
            nc.sync.dma_start(wk_sb[:], wk_d.rearrange("(a p) n -> p a n", p=128))
            wv_sb = cp.tile([128, 6, HPC * 65], bf16)
            nc.sync.dma_start(wv_sb[:], wv_d.rearrange("(a p) n -> p a n", p=128))
            bv_sb = cp.tile([1, HPC * 65], bf16)
            nc.sync.dma_start(bv_sb[:], bv_d)
            wco_lo = cp.tile([128, D], bf16)
            nc.sync.dma_start(wco_lo[:], wco_d[0:128, :])
            wco_hi = cp.tile([65, D], bf16)
            nc.sync.dma_start(wco_hi[:], wco_d[128:193, :])
            bq_lo = cp.tile([128, 1], f32)
            nc.sync.dma_start(bq_lo[:, 0], bq_d[0:128])
            bq_hi = cp.tile([64, 1], f32)
            nc.sync.dma_start(bq_hi[:, 0], bq_d[128:192])
            bk_lo = cp.tile([128, 1], f32)
            nc.sync.dma_start(bk_lo[:, 0], bk_d[0:128])
            bk_hi = cp.tile([64, 1], f32)
            nc.sync.dma_start(bk_hi[:, 0], bk_d[128:192])
            mk_sb = cp.tile([KT, n_blocks, QP], bf16)
            nc.sync.dma_start(mk_sb[:], mk_d.rearrange("n p q -> p n q"))
            ident = cp.tile([128, 128], bf16)
            make_identity(nc, ident[:])
            ones = cp.tile([1, 128], bf16)
            nc.vector.memset(ones[:], 1.0)

            xqT = wp.tile([128, 6, S], bf16)
            xkT = wp.tile([128, 6, S], bf16)
            xvT = wp.tile([128, 6, S], bf16)
            for j in range(6):
                nc.sync.dma_start_transpose(xqT[:, j, :], xq_d[:, 128 * j:128 * (j + 1)])
                nc.sync.dma_start_transpose(xkT[:, j, :], xk_d[:, 128 * j:128 * (j + 1)])
                nc.sync.dma_start_transpose(xvT[:, j, :], xv_d[:, 128 * j:128 * (j + 1)])

            QT_lo = wp.tile([128, S], bf16)
            QT_hi = wp.tile([64, S], bf16)
            KT_lo = wp.tile([128, S], bf16)
            KT_hi = wp.tile([64, S], bf16)
            Vp = wp.tile([128, NKT, HPC * 65], bf16)
            aT_lo = wp.tile([128, S], bf16)
            aT_hi = wp.tile([65, S], bf16)
            nc.vector.memset(aT_hi[64:65, :], 1.0)

            with tc.tile_pool(name="psA", bufs=1, space="PSUM") as psA:
                # ---- Q.T / K.T projections ----
                for w_sb, xT, b_lo, b_hi, d_lo, d_hi in (
                    (wq_sb, xqT, bq_lo, bq_hi, QT_lo, QT_hi),
                    (wk_sb, xkT, bk_lo, bk_hi, KT_lo, KT_hi),
                ):
                    for rows, roff, dst, bias in ((128, 0, d_lo, b_lo),
                                                  (64, 128, d_hi, b_hi)):
                        for s in range(4):
                            sl = slice(512 * s, 512 * (s + 1))
                            pj = psA.tile([128, 512], f32, tag="pj", bufs=2)
                            for j in range(6):
                                nc.tensor.matmul(
                                    pj[0:rows, :],
                                    w_sb[:, j, roff:roff + rows],
                                    xT[:, j, sl],
                                    start=(j == 0), stop=(j == 5))
                            nc.vector.tensor_scalar_add(
                                dst[0:rows, sl], pj[0:rows, :], bias[0:rows, :])

                # ---- V projection (with bias+ones row) ----
                for t in range(NKT):
                    sl = slice(KT * t, KT * (t + 1))
                    pj = psA.tile([128, 512], f32, tag="pj", bufs=2)
                    for j in range(6):
                        nc.tensor.matmul(pj[:, 0:HPC * 65],
                                         xvT[:, j, sl], wv_sb[:, j, :],
                                         start=(j == 0), stop=False)
                    nc.tensor.matmul(pj[:, 0:HPC * 65], ones[0:1, :], bv_sb[:],
                                     start=False, stop=True)
                    nc.vector.tensor_copy(Vp[:, t, :], pj[:, 0:HPC * 65])

                # ---- attention ----
                for hh in range(HPC):
                    if hh < 2:
                        qt_src, kt_src, ro = QT_lo, KT_lo, 64 * hh
                    else:
                        qt_src, kt_src, ro = QT_hi, KT_hi, 0
                    for p in range(NP):
                        qsl = slice(QP * p, QP * (p + 1))
                        lts = live[p]
                        at = psA.tile([65, QP], f32, tag="at", bufs=2)
                        n = len(lts)
                        for g0 in range(0, n, GRP):
                            grp = lts[g0:g0 + GRP]
                            sc = psA.tile([128, GRP * QP], f32, tag="sc", bufs=2)
                            for i, (t, blk) in enumerate(grp):
                                ssl = slice(QP * i, QP * (i + 1))
                                nc.tensor.matmul(
                                    sc[:, ssl],
                                    kt_src[ro:ro + 64, KT * t:KT * (t + 1)],
                                    qt_src[ro:ro + 64, qsl],
                                    start=True, stop=(blk is None))
                                if blk is not None:
                                    nc.tensor.matmul(
                                        sc[:, ssl], ident[:], mk_sb[:, blk, :],
                                        start=False, stop=True)
                            pt = fp.tile([128, GRP * QP], bf16, tag="pt")
                            nc.scalar.activation(
                                pt[:, 0:QP * len(grp)], sc[:, 0:QP * len(grp)],
                                Exp, scale=1.0 / np.sqrt(DH))
                            for i, (t, blk) in enumerate(grp):
                                nc.tensor.matmul(
                                    at[:],
                                    Vp[:, t, 65 * hh:65 * (hh + 1)],
                                    pt[:, QP * i:QP * (i + 1)],
                                    start=(g0 + i == 0), stop=(g0 + i == n - 1))
                        # softmax tail: scale rows 0..63 by 1/rowsum (row 64)
                        rcp = fp.tile([1, QP], bf16, tag="rcp")
                        with nc.allow_low_precision(reason="softmax recip bf16"):
                            nc.vector.reciprocal(rcp[:], at[64:65, :])
                        rb = psA.tile([64, QP], f32, tag="pj", bufs=2)
                        nc.tensor.matmul(rb[:], ones[0:1, 0:64], rcp[:],
                                         start=True, stop=True)
                        rbs = fp.tile([64, QP], bf16, tag="rbs")
                        nc.vector.tensor_copy(rbs[:], rb[:])
                        dst = aT_lo[64 * hh:64 * (hh + 1), qsl] if hh < 2 \
                            else aT_hi[0:64, qsl]
                        nc.vector.tensor_mul(dst, at[0:64, :], rbs[:])

            # ---- output projection ----
            with tc.tile_pool(name="psB", bufs=3, space="PSUM") as psB:
                for qt in range(NKT):
                    qsl = slice(KT * qt, KT * (qt + 1))
                    osb = fp.tile([128, D], f32, tag="osb", bufs=2)
                    for c0, cw in ((0, 512), (512, 256)):
                        po = psB.tile([128, 512], f32)
                        nc.tensor.matmul(po[:, 0:cw], aT_lo[:, qsl],
                                         wco_lo[:, c0:c0 + cw],
                                         start=True, stop=False)
                        nc.tensor.matmul(po[:, 0:cw], aT_hi[:, qsl],
                                         wco_hi[:, c0:c0 + cw],
                                         start=False, stop=True)
                        nc.vector.tensor_copy(osb[:, c0:c0 + cw], po[:, 0:cw])
                    nc.sync.dma_start(out_d[qsl, :], osb[:])

    nc.compile()
    return nc


def _make_in_maps(x, mask, wq, bq, wk, bk, wv, bv, wc, bc, blocks):
    x = np.asarray(x, np.float32)
    in_maps = []
    xs = {}
    for b in range(B):
        xs[b] = (x[b, :, 0:D].astype(BF16),
                 x[b, :, D:2 * D].astype(BF16),
                 x[b, :, 2 * D:3 * D].astype(BF16))
    for c in range(NCORES):
        b, hs = c // 4, HPC * (c % 4)
        cs = slice(DH * hs, DH * (hs + HPC))
        wv3i = np.zeros((D, HPC * 65), np.float32)
        bv3i = np.zeros((1, HPC * 65), np.float32)
        for hh in range(HPC):
            wv3i[:, 65 * hh:65 * hh + 64] = wv[:, DH * (hs + hh):DH * (hs + hh + 1)]
            bv3i[0, 65 * hh:65 * hh + 64] = bv[DH * (hs + hh):DH * (hs + hh + 1)]
            bv3i[0, 65 * hh + 64] = 1.0  # ones column for the rowsum trick
        wco = np.empty((HPC * DH + 1, D), np.float32)
        wco[0:HPC * DH] = wc[cs, :]
        wco[HPC * DH] = np.asarray(bc, np.float32) / 4.0
        in_maps.append({
            "xq": xs[b][0], "xk": xs[b][1], "xv": xs[b][2],
            "wq3": np.ascontiguousarray(wq[:, cs]).astype(BF16),
            "wk3": np.ascontiguousarray(wk[:, cs]).astype(BF16),
            "wv3i": wv3i.astype(BF16),
            "bq3": np.ascontiguousarray(bq[cs]).astype(np.float32),
            "bk3": np.ascontiguousarray(bk[cs]).astype(np.float32),
            "bv3i": bv3i.astype(BF16),
            "wco": wco.astype(BF16),
            "maskt": blocks,
        })
    return in_maps


_CACHE = {}


def _get_program(live, n_blocks):
    key = (tuple(tuple(l) for l in live), n_blocks)
    if key not in _CACHE:
        _CACHE[key] = _build_program(live, n_blocks)
    return _CACHE[key]


def kernel(x, mask, wq, bq, wk, bk, wv, bv, wc, bc):
    live, blocks = _classify_mask(mask)
    nc = _get_program(live, blocks.shape[0])
    in_maps = _make_in_maps(x, mask, wq, bq, wk, bk, wv, bv, wc, bc, blocks)
    res = run_bass_kernel_spmd(nc, in_maps, list(range(NCORES)))
    if getattr(kernel, "_keep_results", False):
        kernel.last_results = res
    out = np.empty((B, S, D), np.float32)
    for b in range(B):
        acc = res.results[4 * b]["partial"].astype(np.float32).copy()
        for j in range(1, 4):
            acc += res.results[4 * b + j]["partial"]
        out[b] = acc
    return out


# revision 17
# speedup vs baseline: 55.2663x; 55.2663x over previous
# Multi-head causal attention (B=2, S=2048, D=768, H=12) on 8 Trainium2 cores.
#
# Sharding: (batch, head)-parallel. Each core owns 3 heads of one batch and
# computes q/k/v projections for those heads, causal attention, and its
# 3-head slice of the output projection as a partial sum [S, D]. The host
# sums the four partials per batch (the output projection is linear in the
# concatenated heads) -- no on-device collectives needed, and every core runs
# an identical instruction stream (SPMD-clean), with per-core behavior
# carried entirely by the input data.
#
# On-chip layout (all matmul operands bf16, accumulation fp32 in PSUM):
#   x.T tiles arrive via HWDGE DMA-transpose from host-precast bf16 arrays.
#   Q.T,K.T: [dh, s] so scores.T = K.T_tile.T @ Q.T needs no transposes.
#   V': [s, 65*hh] with a ones column per head -> attn.T = V'.T @ P.T gives
#       the softmax denominator as row 64 for free.
#   softmax skips max-subtraction (scores ~ N(0,1); exp is safe in fp32) and
#   is fused into one ScalarE exp op per PSUM group of score tiles.
#   Causal masking: host inspects the runtime mask, classifies 128x256 tiles
#   as dead (skipped), clean, or masked (additive bf16 block applied via an
#   identity-matmul accumulate into the scores PSUM group).

import os
import numpy as np
import ml_dtypes

import concourse.bass as bass
import concourse.bacc as bacc
import concourse.mybir as mybir
import concourse.tile as tile
from concourse.bass_utils import run_bass_kernel_spmd
from concourse.masks import make_identity

BF16 = ml_dtypes.bfloat16
bf16 = mybir.dt.bfloat16
f32 = mybir.dt.float32

B, S, D, H = 2, 2048, 768, 12
DH = D // H            # 64
NCORES = 8
HPC = 3                # heads per core
QP = 256               # q-pair width (softmax column group)
NP = S // QP           # 8 pairs
KT = 128               # key tile
NKT = S // KT          # 16 key tiles
MASK_NEG = -30000.0    # effectively -inf after exp, exact 0.0 in fp32


def _classify_mask(mask):
    """Host-side mask inspection -> per-(pair, ktile) schedule + mask blocks.

    Returns (live, blocks): live[p] is a list of (ktile, block_idx|None);
    blocks is [n, KT, QP] bf16 additive mask blocks (transposed to [k, q]).
    The schedule is global (mask broadcasts over batch/head), so the SPMD
    program is identical on every core.
    """
    m = np.asarray(mask, np.float32).reshape(S, S)  # [q, k]
    # Blocks are added to raw q@k.T scores, *before* the fused exp(x/sqrt(DH))
    # scale, so pre-multiply by sqrt(DH) to match reference's mask * -1e9.
    add = np.clip(m * np.float32(-1e9) * np.float32(np.sqrt(DH)),
                  MASK_NEG, -MASK_NEG).astype(np.float32)
    alive = add > (MASK_NEG + 1.0)
    if not alive.any(axis=1).all():
        # A fully-masked query row: softmax is degenerate; keep every tile
        # live so we at least match reference up to its own degeneracy.
        alive = np.ones_like(alive)
    live = []
    blocks = []
    bkey = {}
    for p in range(NP):
        qs = slice(QP * p, QP * (p + 1))
        lst = []
        for t in range(NKT):
            ks = slice(KT * t, KT * (t + 1))
            if not alive[qs, ks].any():
                continue  # dead tile: every entry masked away
            blk = add[qs, ks]
            if np.all(blk == 0.0):
                lst.append((t, None))
            else:
                tb = np.ascontiguousarray(blk.T).astype(BF16)  # [k, q]
                key = tb.tobytes()
                if key not in bkey:
                    bkey[key] = len(blocks)
                    blocks.append(tb)
                lst.append((t, bkey[key]))
        assert lst, "pair with no live key tiles"
        live.append(lst)
    if blocks:
        blocks = np.stack(blocks)
    else:
        blocks = np.zeros((1, KT, QP), BF16)
    return live, blocks


def _build_program(live, n_blocks, repeat=1, pack=False, grp=4, out_bf16=True):
    """Emit the SPMD Bass program (same for all cores).

    repeat > 1 re-runs the whole body (including x DMA) that many times --
    used only for timing measurements (slope isolates true kernel time).
    """
    nc = bacc.Bacc("TRN2", target_bir_lowering=False, debug=False,
                   num_devices=NCORES)

    # x slices arrive pre-transposed ([D, S]) and pre-cast to bf16 by the host
    xq_d = nc.dram_tensor("xq", [D, S], bf16, kind="ExternalInput").ap()
    xk_d = nc.dram_tensor("xk", [D, S], bf16, kind="ExternalInput").ap()
    xv_d = nc.dram_tensor("xv", [D, S], bf16, kind="ExternalInput").ap()
    wq_d = nc.dram_tensor("wq3", [D, HPC * DH], bf16, kind="ExternalInput").ap()
    wk_d = nc.dram_tensor("wk3", [D, HPC * DH], bf16, kind="ExternalInput").ap()
    wv_d = nc.dram_tensor("wv3i", [D, HPC * 65], bf16, kind="ExternalInput").ap()
    bq_d = nc.dram_tensor("bq3", [HPC * DH], f32, kind="ExternalInput").ap()
    bk_d = nc.dram_tensor("bk3", [HPC * DH], f32, kind="ExternalInput").ap()
    bv_d = nc.dram_tensor("bv3i", [1, HPC * 65], bf16, kind="ExternalInput").ap()
    wco_d = nc.dram_tensor("wco", [HPC * DH + 1, D], bf16, kind="ExternalInput").ap()
    mk_d = nc.dram_tensor("maskt", [n_blocks, KT, QP], bf16, kind="ExternalInput").ap()
    out_d = nc.dram_tensor("partial", [S, D],
                           bf16 if out_bf16 else f32,
                           kind="ExternalOutput").ap()

    Exp = mybir.ActivationFunctionType.Exp
    GRP = grp  # score ktiles per PSUM exp group
    sc_bufs = 2 if GRP <= 4 else 1
    odt = bf16 if out_bf16 else f32

    with tile.TileContext(nc) as tc:
      with tc.tile_pool(name="const", bufs=1) as cp:
        # ---- constants to SBUF (once) ----
        wq_sb = cp.tile([128, 6, HPC * DH], bf16)
        nc.sync.dma_start(wq_sb[:], wq_d.rearrange("(a p) n -> p a n", p=128))
        wk_sb = cp.tile([128, 6, HPC * DH], bf16)
        nc.sync.dma_start(wk_sb[:], wk_d.rearrange("(a p) n -> p a n", p=128))
        wv_sb = cp.tile([128, 6, HPC * 65], bf16)
        nc.sync.dma_start(wv_sb[:], wv_d.rearrange("(a p) n -> p a n", p=128))
        bv_sb = cp.tile([1, HPC * 65], bf16)
        nc.sync.dma_start(bv_sb[:], bv_d)
        wco_lo = cp.tile([128, D], bf16)
        nc.sync.dma_start(wco_lo[:], wco_d[0:128, :])
        wco_hi = cp.tile([65, D], bf16)
        nc.sync.dma_start(wco_hi[:], wco_d[128:193, :])
        bq_lo = cp.tile([128, 1], f32)
        nc.sync.dma_start(bq_lo[:, 0], bq_d[0:128])
        bq_hi = cp.tile([64, 1], f32)
        nc.sync.dma_start(bq_hi[:, 0], bq_d[128:192])
        bk_lo = cp.tile([128, 1], f32)
        nc.sync.dma_start(bk_lo[:, 0], bk_d[0:128])
        bk_hi = cp.tile([64, 1], f32)
        nc.sync.dma_start(bk_hi[:, 0], bk_d[128:192])
        mk_sb = cp.tile([KT, n_blocks, QP], bf16)
        nc.sync.dma_start(mk_sb[:], mk_d.rearrange("n p q -> p n q"))
        ident = cp.tile([128, 128], bf16)
        make_identity(nc, ident[:])
        ones = cp.tile([1, 128], bf16)
        nc.vector.memset(ones[:], 1.0)

        for rep in range(repeat):
          with tc.tile_pool(name=f"work{rep}", bufs=1) as wp, \
               tc.tile_pool(name=f"flow{rep}", bufs=3) as fp:
            xqT = wp.tile([128, 6, S], bf16)
            xkT = wp.tile([128, 6, S], bf16)
            xvT = wp.tile([128, 6, S], bf16)
            for j in range(6):
                nc.sync.dma_start(xqT[:, j, :], xq_d[128 * j:128 * (j + 1), :])
                nc.sync.dma_start(xkT[:, j, :], xk_d[128 * j:128 * (j + 1), :])
                nc.sync.dma_start(xvT[:, j, :], xv_d[128 * j:128 * (j + 1), :])

            QT_lo = wp.tile([128, S], bf16)
            QT_hi = wp.tile([64, S], bf16)
            KT_lo = wp.tile([128, S], bf16)
            KT_hi = wp.tile([64, S], bf16)
            Vp = wp.tile([128, NKT, HPC * 65], bf16)
            aT_lo = wp.tile([128, S], bf16)
            aT_hi = wp.tile([65, S], bf16)
            nc.vector.memset(aT_hi[64:65, :], 1.0)

            with tc.tile_pool(name=f"psA{rep}", bufs=1, space="PSUM") as psA:
                # ---- Q.T / K.T projections ----
                for w_sb, xT, b_lo, b_hi, d_lo, d_hi in (
                    (wq_sb, xqT, bq_lo, bq_hi, QT_lo, QT_hi),
                    (wk_sb, xkT, bk_lo, bk_hi, KT_lo, KT_hi),
                ):
                    for rows, roff, dst, bias in ((128, 0, d_lo, b_lo),
                                                  (64, 128, d_hi, b_hi)):
                        for s in range(4):
                            sl = slice(512 * s, 512 * (s + 1))
                            pj = psA.tile([128, 512], f32, tag="pj", bufs=2)
                            for j in range(6):
                                nc.tensor.matmul(
                                    pj[0:rows, :],
                                    w_sb[:, j, roff:roff + rows],
                                    xT[:, j, sl],
                                    start=(j == 0), stop=(j == 5))
                            nc.vector.tensor_scalar_add(
                                dst[0:rows, sl], pj[0:rows, :], bias[0:rows, :])

                # ---- V projection (with bias+ones row) ----
                for t in range(NKT):
                    sl = slice(KT * t, KT * (t + 1))
                    pj = psA.tile([128, 512], f32, tag="pj", bufs=2)
                    for j in range(6):
                        nc.tensor.matmul(pj[:, 0:HPC * 65],
                                         xvT[:, j, sl], wv_sb[:, j, :],
                                         start=(j == 0), stop=False)
                    nc.tensor.matmul(pj[:, 0:HPC * 65], ones[0:1, :], bv_sb[:],
                                     start=False, stop=True)
                    nc.vector.tensor_copy(Vp[:, t, :], pj[:, 0:HPC * 65])

                # ---- attention ----
                # one [65, 1024] accumulator per pair: 3 heads side by side
                # (cols 256*hh) -> rowsum row shared -> single recip/bcast.
                for p in range(NP):
                    qsl = slice(QP * p, QP * (p + 1))
                    lts = live[p]
                    n = len(lts)
                    at = psA.tile([65, 4 * QP], f32, tag="at", bufs=1)
                    for hh in range(HPC):
                        if hh < 2:
                            qt_src, kt_src, ro = QT_lo, KT_lo, 64 * hh
                        else:
                            qt_src, kt_src, ro = QT_hi, KT_hi, 0
                        asl = slice(QP * hh, QP * (hh + 1))
                        for g0 in range(0, n, GRP):
                            grp = lts[g0:g0 + GRP]
                            sc = psA.tile([128, GRP * QP], f32, tag="sc", bufs=sc_bufs)
                            for i, (t, blk) in enumerate(grp):
                                ssl = slice(QP * i, QP * (i + 1))
                                if pack:
                                    for kh in (0, 1):
                                        nc.tensor.matmul(
                                            sc[64 * kh:64 * (kh + 1), ssl],
                                            kt_src[ro:ro + 64, KT * t + 64 * kh:
                                                   KT * t + 64 * (kh + 1)],
                                            qt_src[ro:ro + 64, qsl],
                                            start=True, stop=(blk is None),
                                            tile_position=(ro, 64 * kh),
                                            skip_group_check=True)
                                else:
                                    nc.tensor.matmul(
                                        sc[:, ssl],
                                        kt_src[ro:ro + 64, KT * t:KT * (t + 1)],
                                        qt_src[ro:ro + 64, qsl],
                                        start=True, stop=(blk is None))
                                if blk is not None:
                                    nc.tensor.matmul(
                                        sc[:, ssl], ident[:], mk_sb[:, blk, :],
                                        start=False, stop=True,
                                        skip_group_check=pack)
                            pt = fp.tile([128, GRP * QP], bf16, tag="pt")
                            nc.scalar.activation(
                                pt[:, 0:QP * len(grp)], sc[:, 0:QP * len(grp)],
                                Exp, scale=1.0 / np.sqrt(DH))
                            for i, (t, blk) in enumerate(grp):
                                nc.tensor.matmul(
                                    at[:, asl],
                                    Vp[:, t, 65 * hh:65 * (hh + 1)],
                                    pt[:, QP * i:QP * (i + 1)],
                                    start=(g0 + i == 0), stop=(g0 + i == n - 1))
                    # batched softmax tail for all 3 heads of this pair
                    rcp = fp.tile([1, HPC * QP], bf16, tag="rcp")
                    with nc.allow_low_precision(reason="softmax recip bf16"):
                        nc.vector.reciprocal(rcp[:], at[64:65, 0:HPC * QP])
                    rb1 = psA.tile([64, 512], f32, tag="pj", bufs=2)
                    nc.tensor.matmul(rb1[:], ones[0:1, 0:64], rcp[:, 0:512],
                                     start=True, stop=True)
                    rb2 = psA.tile([64, 256], f32, tag="pj", bufs=2)
                    nc.tensor.matmul(rb2[:], ones[0:1, 0:64], rcp[:, 512:768],
                                     start=True, stop=True)
                    rbs = fp.tile([64, HPC * QP], bf16, tag="rbs")
                    nc.vector.tensor_copy(rbs[:, 0:512], rb1[:])
                    nc.vector.tensor_copy(rbs[:, 512:768], rb2[:])
                    for hh in range(HPC):
                        asl = slice(QP * hh, QP * (hh + 1))
                        dst = aT_lo[64 * hh:64 * (hh + 1), qsl] if hh < 2 \
                            else aT_hi[0:64, qsl]
                        nc.vector.tensor_mul(dst, at[0:64, asl], rbs[:, asl])

            # ---- output projection ----
            with tc.tile_pool(name=f"psB{rep}", bufs=3, space="PSUM") as psB:
                for qt in range(NKT):
                    qsl = slice(KT * qt, KT * (qt + 1))
                    osb = fp.tile([128, D], odt, tag="osb", bufs=2)
                    for c0, cw in ((0, 512), (512, 256)):
                        po = psB.tile([128, 512], f32)
                        nc.tensor.matmul(po[:, 0:cw], aT_lo[:, qsl],
                                         wco_lo[:, c0:c0 + cw],
                                         start=True, stop=False)
                        nc.tensor.matmul(po[:, 0:cw], aT_hi[:, qsl],
                                         wco_hi[:, c0:c0 + cw],
                                         start=False, stop=True)
                        if c0 == 0:
                            nc.vector.tensor_copy(osb[:, c0:c0 + cw], po[:, 0:cw])
                        else:
                            nc.scalar.copy(osb[:, c0:c0 + cw], po[:, 0:cw])
                    nc.sync.dma_start(out_d[qsl, :], osb[:])

    nc.compile()
    return nc


def _make_in_maps(x, mask, wq, bq, wk, bk, wv, bv, wc, bc, blocks):
    x = np.asarray(x, np.float32)
    in_maps = []
    xs = {}
    for b in range(B):
        xs[b] = tuple(
            np.ascontiguousarray(x[b, :, i * D:(i + 1) * D].T).astype(BF16)
            for i in range(3))
    for c in range(NCORES):
        b, hs = c // 4, HPC * (c % 4)
        cs = slice(DH * hs, DH * (hs + HPC))
        wv3i = np.zeros((D, HPC * 65), np.float32)
        bv3i = np.zeros((1, HPC * 65), np.float32)
        for hh in range(HPC):
            wv3i[:, 65 * hh:65 * hh + 64] = wv[:, DH * (hs + hh):DH * (hs + hh + 1)]
            bv3i[0, 65 * hh:65 * hh + 64] = bv[DH * (hs + hh):DH * (hs + hh + 1)]
            bv3i[0, 65 * hh + 64] = 1.0  # ones column for the rowsum trick
        wco = np.empty((HPC * DH + 1, D), np.float32)
        wco[0:HPC * DH] = wc[cs, :]
        wco[HPC * DH] = np.asarray(bc, np.float32) / 4.0
        in_maps.append({
            "xq": xs[b][0], "xk": xs[b][1], "xv": xs[b][2],
            "wq3": np.ascontiguousarray(wq[:, cs]).astype(BF16),
            "wk3": np.ascontiguousarray(wk[:, cs]).astype(BF16),
            "wv3i": wv3i.astype(BF16),
            "bq3": np.ascontiguousarray(bq[cs]).astype(np.float32),
            "bk3": np.ascontiguousarray(bk[cs]).astype(np.float32),
            "bv3i": bv3i.astype(BF16),
            "wco": wco.astype(BF16),
            "maskt": blocks,
        })
    return in_maps


_CACHE = {}


def _get_program(live, n_blocks):
    key = (tuple(tuple(l) for l in live), n_blocks)
    if key not in _CACHE:
        _CACHE[key] = _build_program(live, n_blocks)
    return _CACHE[key]


def kernel(x, mask, wq, bq, wk, bk, wv, bv, wc, bc):
    live, blocks = _classify_mask(mask)
    nc = _get_program(live, blocks.shape[0])
    in_maps = _make_in_maps(x, mask, wq, bq, wk, bk, wv, bv, wc, bc, blocks)
    res = run_bass_kernel_spmd(nc, in_maps, list(range(NCORES)))
    if getattr(kernel, "_keep_results", False):
        kernel.last_results = res
    out = np.empty((B, S, D), np.float32)
    for b in range(B):
        acc = res.results[4 * b]["partial"].astype(np.float32).copy()
        for j in range(1, 4):
            acc += res.results[4 * b + j]["partial"]
        out[b] = acc
    return out
